# revision 1
# baseline (speedup 1.0000x reference)
"""MoE grouped-MLP (Megatron GroupedMLP fwd, no gate) on 8 TRN2 NeuronCores.

Strategy: pair experts (largest token-count with smallest) onto 4 core-pairs.
Within a pair, both cores process ALL of the pair's tokens but each takes one
half of the FFN dimension (w1 column-split / w2 row-split).  Each core emits a
partial fc2 output (its half of the F contraction); the host sums the two
partials per pair.  Token tiles are 512 wide; segment A (big expert) and
segment B (small expert) are padded to a uniform tile count so one SPMD
program serves all 8 cores.

All matmuls run transposed (fc1^T = w1^T @ x^T, out^T = w2^T @ act^T) so both
weight operands load in their natural [K, M] layouts and no on-device
transposes are needed.  Inputs are cast to bf16 on the host (PSUM accumulates
fp32); the output returns in fp32.
"""

import math
from contextlib import ExitStack

import ml_dtypes
import numpy as np

import concourse.bass as bass
import concourse.mybir as mybir
import concourse.tile as tile
from concourse import bacc
from concourse.bass_utils import run_bass_kernel_spmd

NTILE = 512  # token tile (moving-operand free dim; one fp32 PSUM bank)
BF16 = mybir.dt.bfloat16
F32 = mybir.dt.float32
NP_BF16 = ml_dtypes.bfloat16

_NC_CACHE = {}


def _build(nt_a, nt_b, h, fh, repeat=1):
    """Trace the SPMD bass program: one core's share of the paired-expert MLP.

    nt_a/nt_b: number of 512-token tiles in segment A (big expert) and B.
    h: hidden size.  fh: this core's share of the ffn dim (F/2).
    repeat: unroll the whole token loop R times (benchmarking only — lets a
    timing harness take a slope over R to cancel fixed dispatch overheads).
    """
    key = (nt_a, nt_b, h, fh, repeat)
    if key in _NC_CACHE:
        return _NC_CACHE[key]

    nt = nt_a + nt_b
    p_tok = nt * NTILE
    kh = h // 128    # fc1 contraction tiles
    kf = fh // 128   # fc2 contraction tiles
    m1 = fh // 128   # fc1 output partition tiles
    m2 = h // 128    # fc2 output partition tiles

    nc = bacc.Bacc()
    xT = nc.dram_tensor("xT", [h, p_tok], BF16, kind="ExternalInput")
    w1h = nc.dram_tensor("w1h", [2, h, fh], BF16, kind="ExternalInput")
    w2h = nc.dram_tensor("w2h", [2, fh, h], BF16, kind="ExternalInput")
    outT = nc.dram_tensor("outT", [h, p_tok], F32, kind="ExternalOutput")

    with tile.TileContext(nc) as tc, ExitStack() as ctx:
        wpool = ctx.enter_context(tc.tile_pool(name="weights", bufs=1))
        xpool = ctx.enter_context(tc.tile_pool(name="x", bufs=2))
        apool = ctx.enter_context(tc.tile_pool(name="act", bufs=2))
        opool = ctx.enter_context(tc.tile_pool(name="out", bufs=4))
        ps1 = ctx.enter_context(tc.tile_pool(name="ps1", bufs=4, space="PSUM"))
        ps2 = ctx.enter_context(tc.tile_pool(name="ps2", bufs=4, space="PSUM"))

        # Resident weights, issued in first-use order (seg0 fc1, seg0 fc2, ...).
        w1_sb, w2_sb = {}, {}
        for seg in range(2):
            for k in range(kh):
                t = wpool.tile([128, fh], BF16, name=f"w1_{seg}_{k}")
                nc.sync.dma_start(out=t, in_=w1h[seg, 128 * k : 128 * (k + 1), :])
                w1_sb[seg, k] = t
            for k in range(kf):
                t = wpool.tile([128, h], BF16, name=f"w2_{seg}_{k}")
                nc.sync.dma_start(out=t, in_=w2h[seg, 128 * k : 128 * (k + 1), :])
                w2_sb[seg, k] = t

        for n in range(nt * repeat):
            n = n % nt
            seg = 0 if n < nt_a else 1
            cs = bass.ds(NTILE * n, NTILE)

            x_n = []
            for k in range(kh):
                xt = xpool.tile([128, NTILE], BF16, name=f"x_{k}", tag=f"x{k}")
                # gpsimd (SWDGE): slot-reuse WAR + queue-throttle waits exceed
                # the single wait slot of the HWDGE direct2d instruction.
                nc.gpsimd.dma_start(out=xt, in_=xT[128 * k : 128 * (k + 1), cs])
                x_n.append(xt)

            act_n = []
            for m in range(m1):
                ps = ps1.tile([128, NTILE], F32, name="fc1ps", tag="fc1ps")
                for k in range(kh):
                    nc.tensor.matmul(
                        ps,
                        w1_sb[seg, k][:, 128 * m : 128 * (m + 1)],
                        x_n[k],
                        start=(k == 0),
                        stop=(k == kh - 1),
                    )
                a = apool.tile([128, NTILE], BF16, name=f"a_{m}", tag=f"a{m}")
                nc.scalar.activation(a, ps, mybir.ActivationFunctionType.Gelu)
                act_n.append(a)

            for m in range(m2):
                ps = ps2.tile([128, NTILE], F32, name="fc2ps", tag="fc2ps")
                for k in range(kf):
                    nc.tensor.matmul(
                        ps,
                        w2_sb[seg, k][:, 128 * m : 128 * (m + 1)],
                        act_n[k],
                        start=(k == 0),
                        stop=(k == kf - 1),
                    )
                o = opool.tile([128, NTILE], F32, name="o", tag="o")
                nc.vector.tensor_copy(o, ps)
                nc.gpsimd.dma_start(out=outT[128 * m : 128 * (m + 1), cs], in_=o)

    nc.compile()  # bacc legalization: splits multi-wait DMAs for TRN2 codegen
    _NC_CACHE[key] = nc
    return nc


def _plan(tokens_per_expert):
    """Pair experts big-with-small; return pairs + uniform tile counts."""
    tpe = np.asarray(tokens_per_expert, dtype=np.int64)
    e = len(tpe)
    order = np.argsort(-tpe, kind="stable")
    pairs = [(int(order[i]), int(order[e - 1 - i])) for i in range(e // 2)]
    nt_a = max(1, max(math.ceil(int(tpe[a]) / NTILE) for a, _ in pairs))
    nt_b = max(1, max(math.ceil(int(tpe[b]) / NTILE) for _, b in pairs))
    return tpe, pairs, nt_a, nt_b


def prepare(dispatched_input, tokens_per_expert, w1, w2):
    """Build (nc, in_maps, gather) for the paired-expert SPMD program."""
    t_tot, h = dispatched_input.shape
    e, _, f = w1.shape
    fh = f // 2
    tpe, pairs, nt_a, nt_b = _plan(tokens_per_expert)
    offs = np.concatenate([[0], np.cumsum(tpe)])
    p_tok = (nt_a + nt_b) * NTILE

    nc = _build(nt_a, nt_b, h, fh)

    x_bf = dispatched_input.astype(NP_BF16)
    in_maps = []
    for pi, (ea, eb) in enumerate(pairs):
        # Shared token block for the pair: [h, p_tok] bf16, segments padded.
        xT = np.zeros((h, p_tok), dtype=NP_BF16)
        na, nb = int(tpe[ea]), int(tpe[eb])
        xT[:, :na] = x_bf[offs[ea] : offs[ea] + na].T
        xT[:, nt_a * NTILE : nt_a * NTILE + nb] = x_bf[offs[eb] : offs[eb] + nb].T
        for half in range(2):
            fs = slice(half * fh, (half + 1) * fh)
            w1h = np.stack([w1[ea][:, fs], w1[eb][:, fs]]).astype(NP_BF16)
            w2h = np.stack([w2[ea][fs, :], w2[eb][fs, :]]).astype(NP_BF16)
            in_maps.append({"xT": xT, "w1h": w1h, "w2h": w2h})

    def gather(per_core_out):
        out = np.empty((t_tot, h), dtype=np.float32)
        for pi, (ea, eb) in enumerate(pairs):
            acc = per_core_out[2 * pi] + per_core_out[2 * pi + 1]
            na, nb = int(tpe[ea]), int(tpe[eb])
            out[offs[ea] : offs[ea] + na] = acc[:, :na].T
            out[offs[eb] : offs[eb] + nb] = (
                acc[:, nt_a * NTILE : nt_a * NTILE + nb].T
            )
        return out

    return nc, in_maps, gather


def kernel(dispatched_input, tokens_per_expert, w1, w2, _spmd_kwargs=None):
    nc, in_maps, gather = prepare(dispatched_input, tokens_per_expert, w1, w2)
    res = run_bass_kernel_spmd(
        nc, in_maps, core_ids=list(range(8)), **(_spmd_kwargs or {})
    )
    global LAST_RESULT
    LAST_RESULT = res
    return gather([r["outT"] for r in res.results])

